# revision 87
# baseline (speedup 1.0000x reference)
"""Trainium2 Bass kernel for nn_MixtureOfAdapter (moe_routing).

Math (per token, H=1024, F=256, D=3 domains):
    mu, sd (ddof=1) over H;  s = sd + eps;  xn = (x - mu)/s
    h_d   = xn*g_d + b_d
    mid_d = relu(W1_d h_d + b1_d);  a_d = W2_d mid_d + b2_d
    gate_d = sigmoid(gu_d.x + gv_d.a_d + gb_d)
    out = 2x + sum_d gate_d * a_d

Kernel strategy (8 cores, data-parallel over batch B=8):
  - All matmul-land tensors are bf16: PE transposes run at 1 cyc/row
    (vs 2 for f32) and weights/activations halve SBUF + DMA traffic.
  - Work in normalized-transposed land: per 512-token macro-tile the
    normalized xn = (x-mu)/s (one Act Identity / DVE tensor_scalar op
    with per-partition scale=1/s, bias=-mu/s, alternating engines) is
    transposed to [h, t] via PE identity matmuls; a [128, 33] (mu, s)
    pack is transposed the same way into [1, t] rows at partitions
    0/32 for rank-1 corrections.
  - M1: mid = relu(W1g @ xn^T (+ b1e per-partition bias)) with
    W1g = W1 * ln_g folded host-side.  True mid (no s scaling).
  - Gates: pgv[d,t] = w2gv_d . mid_d (+ mu-row rank-1 for gu.x's mean
    term); pgux[d,t] = gu_d . xn^T; z = pgux*s + pgv;
    gate = sigmoid(z + (gb_d + gv_d.b2_d)).  s broadcast to 3
    partitions via a ones3 rank-1 matmul.
  - gate rows broadcast to 128 partitions via one-hot matmuls; gmid =
    mid * gate (bf16, 2x DVE); M2 accumulates all domains into one
    PSUM in natural [t, h] layout (+ gate-row rank-1 if b2 nonzero);
    out = 2x + pout via one DVE scalar_tensor_tensor per 512-chunk.
  - DMA discipline (the TimelineSim serializes all transfers on one
    DMA_ENGINES resource, rotates 8 HWDGE + 8 SWDGE completion-sem
    lanes in stream order, and charges ~630ns of HWDGE descriptor gen
    per DMA): few, large DMAs; startup loads (x macro-tile 0 per
    sub-tile + whole w1g) on the sync queue; steady x loads as one
    DMA per macro-tile on the SWDGE (Pool) queue, paced behind the
    previous tile's arrival by a 1-element gating copy; w2t gated
    behind macro-tile 2's load; outputs on SWDGE except the last
    macro-tile's, which go as halves on the idle sync queue.
  - Software-pipelined emission keeps each macro-tile's gate chain
    (DVE/Act latency) hidden behind the next tile's M1 in the PE
    FIFO; macro-tile 0's M1 is emitted in 128-token column slices so
    the PE starts as soon as the first sub-tile's transpose lands.
  - _split_multiwaits rewrites >1-wait instructions (walrus limit)
    to park extra waits on Memset/Copy carrier ops that wait in the
    engine wait queue instead of NoOps that would stall the
    sequencer.
"""

import numpy as np

import concourse.bass as bass
import concourse.mybir as mybir
import concourse.tile as tile
from concourse.bass_utils import run_bass_kernel_spmd

B, L, H, F, D = 8, 2048, 1024, 256, 3
EPS = 1e-6
T = 512                 # tokens per macro-tile
NSUB = T // 128         # 4 sub-tiles of 128 tokens
NMT = L // T            # 4 macro-tiles per core
KCH = H // 128          # 8 k-chunks over H
FCH = (D * F) // 128    # 6 chunks over stacked (domain, F)
NCH = H // 512          # 2 output column chunks
DF = D * F

f32 = mybir.dt.float32
bf16 = mybir.dt.bfloat16
AF = mybir.ActivationFunctionType
ALU = mybir.AluOpType


def _split_multiwaits(nc):
    """This walrus build allows 1 sync-wait per instruction (2 for
    EventSemaphore); Tile can attach more.  Move extras onto preceding
    same-engine carrier instructions.  A bare NoOp holds the sequencer
    while it waits (stalling dispatch of everything behind it), so where
    possible the carrier is a 1-element Memset to a dead scratch column:
    a real engine instruction parks its wait in the engine wait queue
    and lets the sequencer keep dispatching."""
    import copy
    tmpl = {}
    for f in nc.m.functions:
        for bb in f.blocks:
            for inst in bb.instructions:
                if (isinstance(inst, mybir.InstMemset)
                        and inst.engine not in tmpl):
                    tmpl[inst.engine] = inst
                elif (isinstance(inst, mybir.InstActivation)
                        and inst.func == AF.Copy
                        and inst.engine not in tmpl):
                    tmpl[inst.engine] = inst

    def carrier(inst, w, j):
        t = tmpl.get(inst.engine)
        if t is not None:
            c = copy.deepcopy(t)
            c.name = f"{inst.name}-wsplit{j}"
            c.sync_info = mybir.SyncInfo(on_wait=[w], on_update=[])
            return c
        return mybir.InstNoOp(
            name=f"{inst.name}-wsplit{j}",
            engine=inst.engine,
            sync_info=mybir.SyncInfo(on_wait=[w], on_update=[]),
            ins=[], outs=[],
        )

    for f in nc.m.functions:
        for bb in f.blocks:
            new = []
            changed = False
            for inst in bb.instructions:
                si = inst.sync_info
                cap = 2 if isinstance(inst, mybir.InstEventSemaphore) else 1
                if si is not None and len(si.on_wait) > cap:
                    waits = list(si.on_wait)
                    extra, kept = waits[:-cap], waits[-cap:]
                    for j, w in enumerate(extra):
                        new.append(carrier(inst, w, j))
                    inst.sync_info = mybir.SyncInfo(
                        on_wait=kept, on_update=list(si.on_update))
                    changed = True
                new.append(inst)
            if changed:
                bb.instructions = new


def _build(has_b1e: bool, has_b2: bool):
    nc = bass.Bass(target_bir_lowering=False)

    xin = nc.dram_tensor("xin", [L, H], f32, kind="ExternalInput")
    w1g = nc.dram_tensor("w1g", [128, KCH, DF], bf16, kind="ExternalInput")
    w2t = nc.dram_tensor("w2t", [128, FCH, H], bf16, kind="ExternalInput")
    cpb = nc.dram_tensor("cpb", [128, 560], bf16, kind="ExternalInput")
    cpf = nc.dram_tensor("cpf", [128, 8], f32, kind="ExternalInput")
    if has_b2:
        b2r = nc.dram_tensor("b2r", [D, H], bf16, kind="ExternalInput")
    out = nc.dram_tensor("out", [L, H], f32, kind="ExternalOutput")

    # [L, H] viewed as [128p, sub, H] per macro-tile
    x_mt = xin.ap().rearrange("(m s p) h -> m p s h", p=128, s=NSUB)
    out_mt = out.ap().rearrange("(m s p) h -> m p s h", p=128, s=NSUB)

    with tile.TileContext(nc) as tc:
        with (
            tc.tile_pool(name="const", bufs=1) as const,
            tc.tile_pool(name="xp", bufs=3) as xp,
            tc.tile_pool(name="xnp", bufs=3) as xnp,
            tc.tile_pool(name="xtp", bufs=3) as xtp,
            tc.tile_pool(name="midp", bufs=3) as midp,
            tc.tile_pool(name="gmp", bufs=3) as gmp,
            tc.tile_pool(name="gbp", bufs=3) as gbp,
            tc.tile_pool(name="outp", bufs=4) as outp,
            tc.tile_pool(name="smalls", bufs=6) as smalls,
            tc.tile_pool(name="gsm", bufs=3) as gsm,
            tc.tile_pool(name="ps_m1", bufs=2, space="PSUM") as ps_m1,
            tc.tile_pool(name="ps_m2", bufs=2, space="PSUM") as ps_m2,
            tc.tile_pool(name="ps_gux", bufs=1, space="PSUM") as ps_gux,
            tc.tile_pool(name="ps_gv", bufs=1, space="PSUM") as ps_gv,
            tc.tile_pool(name="ps_tr", bufs=2, space="PSUM") as ps_tr,
        ):
            # scratch columns for multiwait carrier ops (dead stores; the
            # first Memset per engine / Copy activation becomes the carrier
            # template in _split_multiwaits)
            scratch = const.tile([128, 4], f32)
            nc.vector.memset(scratch[:, 0:1], 0.0)
            nc.gpsimd.memset(scratch[:, 2:3], 0.0)
            nc.scalar.copy(scratch[0:1, 1:2], scratch[0:1, 3:4])

            # constants on scalar queue (small, needed early); x + weights
            # on sync queue, x sub-tile 0 first, each weight pack one DMA
            # (HWDGE descriptor gen costs ~630ns per DMA instruction)
            cpb_sb = const.tile([128, 560], bf16)
            cpf_sb = const.tile([128, 8], f32)
            nc.scalar.dma_start(out=cpb_sb, in_=cpb.ap())
            nc.scalar.dma_start(out=cpf_sb, in_=cpf.ap())
            oh_sb = cpb_sb[0:D, 0:384]
            gus_sb = cpb_sb[:, 384:408].rearrange("p (k d) -> p k d", d=D)
            w2gv_sb = cpb_sb[:, 408:426].rearrange("p (c d) -> p c d", d=D)
            gusum_sb = cpb_sb[0:1, 426:429]
            ones3_sb = cpb_sb[32:33, 429:432]
            ident_b = cpb_sb[:, 432:560]
            gb3_sb = cpf_sb[0:D, 6:7]

            x_first = xp.tile([128, NSUB, H], f32, tag="x")
            w1g_sb = const.tile([128, KCH, DF], bf16)
            w2t_sb = const.tile([128, FCH, H], bf16)
            nc.sync.dma_start(out=x_first[:, 0, :], in_=x_mt[0][:, 0, :])
            nc.sync.dma_start(out=x_first[:, 1, :], in_=x_mt[0][:, 1, :])
            nc.sync.dma_start(out=w1g_sb.rearrange("p a b -> p (a b)"),
                              in_=w1g.ap().rearrange("p a b -> p (a b)"))
            nc.sync.dma_start(out=x_first[:, 2, :], in_=x_mt[0][:, 2, :])
            nc.sync.dma_start(out=x_first[:, 3, :], in_=x_mt[0][:, 3, :])
            if has_b2:
                b2r_sb = const.tile([D, H], bf16)
                nc.scalar.dma_start(out=b2r_sb, in_=b2r.ap())

            def stage_load(mt, x_pre=None, prev_x=None):
                """x load only (emitted early for DMA pacing)."""
                if x_pre is not None:
                    x_t = x_pre
                else:
                    # one DMA per macro-tile keeps the scheduler from
                    # reordering sub-tile arrivals.  mt1 rides the sync
                    # queue right behind the startup weight loads; later
                    # tiles ride the SWDGE (Pool) lanes, paced behind the
                    # previous macro-tile's arrival by a 1-element gating
                    # copy (overwritten by the DMA) so they can't steal
                    # DMA bandwidth from older, more urgent transfers.
                    x_t = xp.tile([128, NSUB, H], f32, tag="x")
                    if prev_x is not None:
                        # gate on sub-tile 2's region: for mt1 (whose
                        # predecessor loads per sub-tile) this releases the
                        # load one x-transfer earlier; for later tiles the
                        # predecessor is one DMA and any region is its
                        # completion
                        nc.gpsimd.tensor_copy(x_t[0:1, 0, 0:1],
                                              prev_x[0:1, 2, 0:1])
                    nc.gpsimd.dma_start(out=x_t, in_=x_mt[mt])
                return x_t

            def stage_a(mt, x_t):
                """stats, normalize (bf16), PE transposes."""
                xn_b = xnp.tile([128, NSUB, H], bf16, tag="xn")
                xnT = xtp.tile([128, KCH, T], bf16, tag="xnT")
                rows_b = xtp.tile([33, T], bf16, tag="rows")
                for ss in range(NSUB):
                    xs = x_t[:, ss, :]
                    tsl = slice(ss * 128, (ss + 1) * 128)
                    st = smalls.tile([128, 2, 6], f32, tag="bnst")
                    nc.vector.bn_stats(out=st[:, 0, :], in_=xs[:, 0:512])
                    nc.vector.bn_stats(out=st[:, 1, :], in_=xs[:, 512:1024])
                    mv = smalls.tile([128, 2], f32, tag="mv")
                    nc.vector.bn_aggr(out=mv, in_=st)
                    # sc: 0=r=1/s, 1=-mu*r, 2=s=sd+eps
                    sc = smalls.tile([128, 4], f32, tag="sc")
                    nc.scalar.activation(out=sc[:, 2:3], in_=mv[:, 1:2],
                                         func=AF.Sqrt,
                                         scale=float(H) / (H - 1))
                    nc.vector.tensor_scalar_add(sc[:, 2:3], sc[:, 2:3], EPS)
                    nc.vector.reciprocal(sc[:, 0:1], sc[:, 2:3])
                    nc.vector.tensor_scalar(out=sc[:, 1:2], in0=mv[:, 0:1],
                                            scalar1=sc[:, 0:1], scalar2=-1.0,
                                            op0=ALU.mult, op1=ALU.mult)
                    # (mu, s) pack for the row transpose: mu -> row 0,
                    # s -> row 32 (matmul base partitions must be 0/32/64)
                    pk = smalls.tile([128, 33], bf16, tag="pk")
                    nc.vector.tensor_copy(pk[:, 0:1], mv[:, 0:1])
                    nc.vector.tensor_copy(pk[:, 32:33], sc[:, 2:3])
                    # xn = x*(1/s) + (-mu/s), one op, bf16 out; alternate
                    # engines so two sub-tile centers run concurrently
                    if ss % 2 == 0:
                        nc.scalar.activation(out=xn_b[:, ss, :], in_=xs,
                                             func=AF.Identity,
                                             scale=sc[:, 0:1],
                                             bias=sc[:, 1:2])
                    else:
                        nc.vector.tensor_scalar(out=xn_b[:, ss, :], in0=xs,
                                                scalar1=sc[:, 0:1],
                                                scalar2=sc[:, 1:2],
                                                op0=ALU.mult, op1=ALU.add)
                    # PE transposes (bf16: 1 cyc/row); psum->sbuf copies
                    # split between Act and DVE to balance load
                    for half in range(2):
                        ptr = ps_tr.tile([128, 512], bf16, tag="tr")
                        for q in range(4):
                            k = half * 4 + q
                            nc.tensor.transpose(
                                ptr[:, q * 128:(q + 1) * 128],
                                xn_b[:, ss, k * 128:(k + 1) * 128], ident_b)
                        dst = xnT[:, half * 4:(half + 1) * 4, tsl]
                        src = ptr.rearrange("p (q t) -> p q t", q=4)
                        if half == 0:
                            nc.scalar.activation(out=dst, in_=src,
                                                 func=AF.Copy)
                        else:
                            nc.vector.tensor_copy(dst, src)
                    ptr2 = ps_tr.tile([33, 128], bf16, tag="tr")
                    nc.tensor.transpose(ptr2, pk, ident_b)
                    nc.scalar.activation(out=rows_b[:, tsl], in_=ptr2,
                                         func=AF.Copy)
                return dict(x_t=x_t, xnT=xnT, rows=rows_b)

            def stage_b(mt, st_, sliced=False):
                """M1: mid = relu(W1g @ xn^T (+ b1e)).  Sliced mode (macro-
                tile 0) runs sub-tile pairs through half-width psums with
                pgux appended, so ALL work for the first two sub-tiles is
                in the PE FIFO before anything that waits on the later
                arrivals."""
                xnT = st_["xnT"]
                mid = midp.tile([128, FCH, T], bf16, tag="mid")
                if sliced:
                    # wave 1: chunks 0-3 sub-tile-major across four psum
                    # tiles (ps_m2 is idle this early), so two sub-tiles'
                    # worth of matmuls queue before anything waits on the
                    # later x arrivals; wave 2: chunks 4-5
                    p1s = []
                    for ci in range(2):
                        pw = ps_m1.tile([128, T], f32, tag="m1",
                                        name=f"m1w{ci}")
                        p1s.append(pw)
                    for ci in range(2):
                        pw = ps_m2.tile([128, 512], f32, tag="m2",
                                        name=f"m2w{ci}")
                        p1s.append(pw)
                    pgux = ps_gux.tile([D, T], f32, tag="gux")
                    st_["pgux"] = pgux
                    for ss in range(NSUB):
                        tsl = slice(ss * 128, (ss + 1) * 128)
                        for c in range(4):
                            w1c = w1g_sb[:, :, c * 128:(c + 1) * 128]
                            for k in range(KCH):
                                nc.tensor.matmul(
                                    p1s[c][:, tsl], w1c[:, k, :],
                                    xnT[:, k, tsl],
                                    start=(k == 0), stop=(k == KCH - 1))
                        if ss == 1:
                            # fill the wait for the later x arrivals with
                            # the gate matmuls for the first two sub-tiles
                            for gss in range(2):
                                gsl = slice(gss * 128, (gss + 1) * 128)
                                for k in range(KCH):
                                    nc.tensor.matmul(
                                        pgux[:, gsl], gus_sb[:, k, :],
                                        xnT[:, k, gsl],
                                        start=(k == 0), stop=(k == KCH - 1))
                    for c in range(4):
                        bias = cpf_sb[:, c:c + 1] if has_b1e else 0.0
                        nc.scalar.activation(out=mid[:, c, :], in_=p1s[c],
                                             func=AF.Relu, bias=bias)
                    for c in range(4, FCH):
                        p1 = ps_m1.tile([128, T], f32, tag="m1")
                        w1c = w1g_sb[:, :, c * 128:(c + 1) * 128]
                        for ss in range(NSUB):
                            tsl = slice(ss * 128, (ss + 1) * 128)
                            for k in range(KCH):
                                nc.tensor.matmul(
                                    p1[:, tsl], w1c[:, k, :], xnT[:, k, tsl],
                                    start=(k == 0), stop=(k == KCH - 1))
                        bias = cpf_sb[:, c:c + 1] if has_b1e else 0.0
                        nc.scalar.activation(out=mid[:, c, :], in_=p1,
                                             func=AF.Relu, bias=bias)
                    st_["mid"] = mid
                    return
                for c in range(FCH):
                    p1 = ps_m1.tile([128, T], f32, tag="m1")
                    w1c = w1g_sb[:, :, c * 128:(c + 1) * 128]
                    for k in range(KCH):
                        nc.tensor.matmul(p1, w1c[:, k, :], xnT[:, k, :],
                                         start=(k == 0),
                                         stop=(k == KCH - 1))
                    bias = cpf_sb[:, c:c + 1] if has_b1e else 0.0
                    nc.scalar.activation(out=mid[:, c, :], in_=p1,
                                         func=AF.Relu, bias=bias)
                st_["mid"] = mid

            def stage_c(mt, st_):
                """Gates + gmid = gate * mid."""
                xnT, mid = st_["xnT"], st_["mid"]
                murow = st_["rows"][0:1, :]
                srow = st_["rows"][32:33, :]
                pgux = st_.get("pgux")
                if pgux is None:
                    pgux = ps_gux.tile([D, T], f32, tag="gux")
                    for k in range(KCH):
                        nc.tensor.matmul(pgux, gus_sb[:, k, :], xnT[:, k, :],
                                         start=(k == 0), stop=(k == KCH - 1))
                else:
                    # sub-tiles 0/1 were accumulated inside the M1 wave
                    for gss in range(2, NSUB):
                        gsl = slice(gss * 128, (gss + 1) * 128)
                        for k in range(KCH):
                            nc.tensor.matmul(pgux[:, gsl], gus_sb[:, k, :],
                                             xnT[:, k, gsl],
                                             start=(k == 0),
                                             stop=(k == KCH - 1))
                pgv = ps_gv.tile([D, T], f32, tag="gv")
                for c in range(FCH):
                    nc.tensor.matmul(pgv, w2gv_sb[:, c, :], mid[:, c, :],
                                     start=(c == 0), stop=False)
                # gu.x = s*(gu.xn) + mu*sum(gu): mu rank-1 joins pgv's psum
                nc.tensor.matmul(pgv, gusum_sb, murow, start=False, stop=True)
                # s broadcast to 3 partitions
                s3_ps = ps_tr.tile([D, T], f32, tag="tr")
                nc.tensor.matmul(s3_ps, ones3_sb, srow, start=True, stop=True)
                s3b = gsm.tile([D, T], bf16, tag="s3")
                nc.scalar.activation(out=s3b, in_=s3_ps, func=AF.Copy)
                z_sb = gsm.tile([D, T], f32, tag="z")
                nc.vector.tensor_tensor(out=z_sb, in0=pgux, in1=s3b,
                                        op=ALU.mult)
                nc.vector.tensor_add(z_sb, z_sb, pgv)
                g_t = gsm.tile([D, T], bf16, tag="g")
                nc.scalar.activation(out=g_t, in_=z_sb, func=AF.Sigmoid,
                                     bias=gb3_sb)
                st_["g_t"] = g_t

            def stage_c2(mt, st_):
                """Gate broadcast + gmid (emitted after the next tile's M1
                so the sigmoid-chain latency never blocks the PE FIFO)."""
                mid, g_t = st_["mid"], st_["g_t"]
                # broadcast gate row d across partitions via one-hot matmul
                gb128 = gbp.tile([128, D, T], bf16, tag="gb")
                for d in range(D):
                    p_b = ps_tr.tile([128, T], f32, tag="tr")
                    nc.tensor.matmul(p_b, oh_sb[:, d * 128:(d + 1) * 128],
                                     g_t, start=True, stop=True)
                    nc.scalar.activation(out=gb128[:, d, :], in_=p_b,
                                         func=AF.Copy)
                gmid = gmp.tile([128, FCH, T], bf16, tag="gmid")
                for c in range(FCH):
                    nc.vector.tensor_mul(gmid[:, c, :], mid[:, c, :],
                                         gb128[:, c // 2, :])
                st_["gmid"] = gmid

            def stage_d(mt, st_):
                """M2 accumulates all domains (+gate*b2) + final out."""
                gmid, x_t = st_["gmid"], st_["x_t"]
                for ss in range(NSUB):
                    tsl = slice(ss * 128, (ss + 1) * 128)
                    out_sb = outp.tile([128, H], f32, tag="osb")
                    for nch in range(NCH):
                        hsl = slice(nch * 512, (nch + 1) * 512)
                        po = ps_m2.tile([128, 512], f32, tag="m2")
                        for c in range(FCH):
                            nc.tensor.matmul(po, gmid[:, c, tsl],
                                             w2t_sb[:, c, hsl],
                                             start=(c == 0),
                                             stop=(c == FCH - 1 and not has_b2))
                        if has_b2:
                            nc.tensor.matmul(po, st_["g_t"][:, tsl],
                                             b2r_sb[:, hsl],
                                             start=False, stop=True)
                        # out = 2*x + pout
                        nc.vector.scalar_tensor_tensor(
                            out=out_sb[:, hsl], in0=x_t[:, ss, hsl],
                            scalar=2.0, in1=po, op0=ALU.mult, op1=ALU.add)
                    if mt == NMT - 1:
                        # last tile: halves on the now-idle sync queue so
                        # the final store pipelines with the last stt
                        for nch in range(NCH):
                            hsl = slice(nch * 512, (nch + 1) * 512)
                            nc.sync.dma_start(out=out_mt[mt][:, ss, hsl],
                                              in_=out_sb[:, hsl])
                    else:
                        # out DMA on the gpsimd (SWDGE) queue: its waits on
                        # the stt can't block x loads (SP) or Act compute
                        nc.gpsimd.dma_start(out=out_mt[mt][:, ss, :],
                                            in_=out_sb)

            # software-pipelined emission: each macro-tile's gate chain
            # overlaps the next tile's M1 in the PE FIFO; mt0's M1 is
            # sliced so the PE starts on the first transposed sub-tile
            S = [None] * NMT
            X = [None] * NMT
            X[0] = stage_load(0, x_pre=x_first)
            S[0] = stage_a(0, X[0])
            stage_b(0, S[0], sliced=True)
            X[1] = stage_load(1, prev_x=X[0])
            S[1] = stage_a(1, X[1])
            stage_c(0, S[0])
            stage_c2(0, S[0])
            stage_b(1, S[1])
            X[2] = stage_load(2, prev_x=X[1])
            # w2t isn't needed until D0: gate its DMA behind mt2's arrival
            # so it can't delay the mt1/mt2 x loads on the transfer engine
            nc.gpsimd.tensor_copy(w2t_sb[0:1, 0, 0:1], X[2][0:1, 0, 0:1])
            nc.sync.dma_start(out=w2t_sb.rearrange("p a b -> p (a b)"),
                              in_=w2t.ap().rearrange("p a b -> p (a b)"))
            S[2] = stage_a(2, X[2])
            stage_d(0, S[0])
            stage_c(1, S[1])
            stage_c2(1, S[1])
            stage_b(2, S[2])
            X[3] = stage_load(3, prev_x=X[2])
            S[3] = stage_a(3, X[3])
            stage_d(1, S[1])
            stage_c(2, S[2])
            stage_c2(2, S[2])
            stage_b(3, S[3])
            stage_c(3, S[3])
            stage_c2(3, S[3])
            stage_d(2, S[2])
            stage_d(3, S[3])

    _split_multiwaits(nc)
    return nc


last_results = None

_built = {}


def _get_nc(has_b1e, has_b2):
    key = (has_b1e, has_b2)
    if key not in _built:
        _built[key] = _build(*key)
    return _built[key]


def _to_bf16(a):
    from ml_dtypes import bfloat16
    return np.asarray(a, dtype=np.float32).astype(bfloat16)


def kernel(x, ln_g, ln_b, W1, b1, W2, b2, gu, gv, gb):
    x = np.asarray(x, dtype=np.float32)
    ln_g = np.asarray(ln_g, dtype=np.float32)
    ln_b = np.asarray(ln_b, dtype=np.float32)
    W1 = np.asarray(W1, dtype=np.float32)
    b1 = np.asarray(b1, dtype=np.float32)
    W2 = np.asarray(W2, dtype=np.float32)
    b2 = np.asarray(b2, dtype=np.float32)
    gu = np.asarray(gu, dtype=np.float32)
    gv = np.asarray(gv, dtype=np.float32)
    gb = np.asarray(gb, dtype=np.float32)

    # ---- host precompute (all small: ~D*F*H) ----
    W1G = W1 * ln_g[:, None, :]                                # [D, F, H]
    b1e = b1 + np.einsum('dfh,dh->df', W1, ln_b)               # [D, F]
    w2gv = np.einsum('dh,dhf->df', gv, W2)                     # [D, F]
    gusum = gu.sum(axis=1)                                     # [D]
    gb_eff = gb + np.einsum('dh,dh->d', gv, b2)                # [D]

    has_b1e = bool(np.any(b1e != 0.0))
    has_b2 = bool(np.any(b2 != 0.0))

    # lhsT for M1: [128, KCH, DF]; col c*128+j = W1G[d(c), fh(c)*128+j, h]
    w1g_in = np.zeros((128, KCH, DF), dtype=np.float32)
    for c in range(FCH):
        d, fh = c // 2, c % 2
        w1g_in[:, :, c * 128:(c + 1) * 128] = (
            W1G[d].T.reshape(KCH, 128, F)[:, :, fh * 128:(fh + 1) * 128]
            .transpose(1, 0, 2))
    # W2 rhs for M2: [128, FCH, H]; w2t[p, c, h] = W2[d, h, fh*128+p]
    w2t_in = np.zeros((128, FCH, H), dtype=np.float32)
    for c in range(FCH):
        d, fh = c // 2, c % 2
        w2t_in[:, c, :] = W2[d, :, fh * 128:(fh + 1) * 128].T

    cpb_in = np.zeros((128, 560), dtype=np.float32)
    for d in range(D):
        cpb_in[d, d * 128:(d + 1) * 128] = 1.0                 # one-hot bcast
    cpb_in[:, 384:408] = np.ascontiguousarray(
        gu.T.reshape(KCH, 128, D).transpose(1, 0, 2)).reshape(128, KCH * D)
    w2gv_in = np.zeros((128, FCH, D), dtype=np.float32)
    for c in range(FCH):
        d, fh = c // 2, c % 2
        w2gv_in[:, c, d] = w2gv[d, fh * 128:(fh + 1) * 128]
    cpb_in[:, 408:426] = w2gv_in.reshape(128, FCH * D)
    cpb_in[0, 426:429] = gusum
    cpb_in[32, 429:432] = 1.0
    cpb_in[:, 432:560] = np.eye(128, dtype=np.float32)         # transpose id

    cpf_in = np.zeros((128, 8), dtype=np.float32)
    if has_b1e:
        for c in range(FCH):
            d, fh = c // 2, c % 2
            cpf_in[:, c] = b1e[d, fh * 128:(fh + 1) * 128]
    cpf_in[0:D, 6] = gb_eff

    nc = _get_nc(has_b1e, has_b2)

    common = {
        "w1g": _to_bf16(w1g_in),
        "w2t": _to_bf16(w2t_in),
        "cpb": _to_bf16(cpb_in),
        "cpf": cpf_in,
    }
    if has_b2:
        common["b2r"] = _to_bf16(b2)
    in_maps = [dict(common, xin=np.ascontiguousarray(x[c]))
               for c in range(B)]
    res = run_bass_kernel_spmd(nc, in_maps, core_ids=list(range(B)))
    global last_results
    last_results = res
    return np.stack([res.results[c]["out"] for c in range(B)])
